# revision 15
# baseline (speedup 1.0000x reference)
"""Trainium2 Bass kernel for nn_Block_82111184765408 (pre-LN transformer block).

B=128, T=256, C=384, H=6, D=64, FF=1536. Data-parallel over batch across 8
NeuronCores (16 batches/core), batches processed in fused pairs (free dim 512).

Dataflow per batch pair (all matmuls fp32r, 1 cycle/row):
  x (token-major) -> LN1 stats on DVE -> h (token-major) -> PE-transpose ->
  h1T (feature-major, LN gain/bias folded into the psum->SBUF copy) ->
  Q^T,K^T (feature-major) / V (token-major, via swapped matmul operands;
  bias added with a ones-row matmul; ones column appended for softmax
  denominators) -> scoresT = Q^T-slices x K^T per head (softmax dim on
  partitions) -> exp on ACT (scale=C^-0.5 folded) -> causal mask multiply on
  DVE -> attn@V with V-hat (denominator lands in psum row 64) ->
  reciprocal + indicator-matmul broadcast -> AVT (feature-major) -> proj ->
  transpose back -> +x residual -> LN2 -> FFN (relu on ACT) -> transpose ->
  residual -> DMA out.
"""

import numpy as np

import concourse.bass as bass
import concourse.mybir as mybir
import concourse.tile as tile
from concourse import bacc
from concourse.bass_utils import run_bass_kernel_spmd
from concourse.masks import make_identity

P = 128
B, T, C, H, D = 128, 256, 384, 6, 64
FF = 4 * C
N_CORES = 8
B_LOCAL = B // N_CORES          # 16 batches per core
N_PAIRS = B_LOCAL // 2          # 8 pairs, free dim 512 per pair
TP = 2 * T                      # 512
CC = C // P                     # 3 feature chunks
FC = FF // P                    # 12 ffn chunks
EPS = 1e-5
SCALE = C ** -0.5

f32 = mybir.dt.float32
f32r = mybir.dt.float32r
AF = mybir.ActivationFunctionType
OP = mybir.AluOpType


def build_nc(n_pairs=N_PAIRS, debug_outputs=False):
    nc = bacc.Bacc("TRN2", target_bir_lowering=False, debug=False)

    x_d = nc.declare_dram_parameter("x", [2 * n_pairs, T, C], f32, isOutput=False)
    ln1_g_d = nc.declare_dram_parameter("ln1_g", [C], f32, isOutput=False)
    ln1_b_d = nc.declare_dram_parameter("ln1_b", [C], f32, isOutput=False)
    Wk_d = nc.declare_dram_parameter("Wk", [H, C, D], f32r, isOutput=False)
    bk_d = nc.declare_dram_parameter("bk", [H, D], f32, isOutput=False)
    Wq_d = nc.declare_dram_parameter("Wq", [H, C, D], f32r, isOutput=False)
    bq_d = nc.declare_dram_parameter("bq", [H, D], f32, isOutput=False)
    Wv_d = nc.declare_dram_parameter("Wv", [H, C, D], f32r, isOutput=False)
    bv_d = nc.declare_dram_parameter("bv", [H, D], f32r, isOutput=False)
    Wp_d = nc.declare_dram_parameter("Wp", [C, C], f32r, isOutput=False)
    bp_d = nc.declare_dram_parameter("bp", [C], f32, isOutput=False)
    ln2_g_d = nc.declare_dram_parameter("ln2_g", [C], f32, isOutput=False)
    ln2_b_d = nc.declare_dram_parameter("ln2_b", [C], f32, isOutput=False)
    W1_d = nc.declare_dram_parameter("W1", [C, FF], f32r, isOutput=False)
    b1_d = nc.declare_dram_parameter("b1", [FF], f32, isOutput=False)
    W2_d = nc.declare_dram_parameter("W2", [FF, C], f32r, isOutput=False)
    b2_d = nc.declare_dram_parameter("b2", [C], f32, isOutput=False)
    y_d = nc.declare_dram_parameter("y", [2 * n_pairs, T, C], f32, isOutput=True)
    dbg = {}
    if debug_outputs:
        for nm, shp in (("den6", [H, TP]), ("dens", [H, TP]), ("E0", [P, TP]),
                        ("QT0", [P, TP]), ("KT0", [P, TP]), ("AVT0", [P, TP]),
                        ("h1T0", [P, TP]), ("V0", [P, TP]), ("AVT1", [P, TP]),
                        ("AVT2", [P, TP]), ("rps0", [P, TP]), ("proj0", [P, TP]),
                        ("out1", [P, 4 * C])):
            dbg[nm] = nc.declare_dram_parameter(nm, shp, f32, isOutput=True)

    with tile.TileContext(nc) as tc:
        with tc.tile_pool(name="const", bufs=1) as cst, \
             tc.tile_pool(name="p2", bufs=2) as p2, \
             tc.tile_pool(name="p1", bufs=1) as p1, \
             tc.tile_pool(name="dr", bufs=2, space="DRAM") as drp, \
             tc.tile_pool(name="ps", bufs=7, space="PSUM") as psp:

            def psum(w=TP, h=P):
                return psp.tile([h, w], f32, tag="ps", name="ps")

            # ---------- constants ----------
            Wq_sb = cst.tile([P, CC, C], f32r, tag="Wq")
            for h in range(H):
                nc.sync.dma_start(Wq_sb[:, :, 64 * h:64 * h + 64],
                                  Wq_d[h].rearrange("(o p) d -> p o d", p=P))
            Wk_sb = cst.tile([P, CC, C], f32r, tag="Wk")
            for h in range(H):
                nc.sync.dma_start(Wk_sb[:, :, 64 * h:64 * h + 64],
                                  Wk_d[h].rearrange("(o p) d -> p o d", p=P))
            Wv_sb = cst.tile([P, CC, C], f32r, tag="Wv")
            for h in range(H):
                nc.sync.dma_start(Wv_sb[:, :, 64 * h:64 * h + 64],
                                  Wv_d[h].rearrange("(o p) d -> p o d", p=P))
            Wp_sb = cst.tile([P, CC, C], f32r, tag="Wp")
            nc.sync.dma_start(Wp_sb[:], Wp_d.rearrange("(o p) c -> p o c", p=P))
            W1_sb = cst.tile([P, CC, FF], f32r, tag="W1")
            nc.sync.dma_start(W1_sb[:], W1_d.rearrange("(o p) f -> p o f", p=P))
            W2_sb = cst.tile([P, FC, C], f32r, tag="W2")
            nc.sync.dma_start(W2_sb[:], W2_d.rearrange("(o p) c -> p o c", p=P))

            g1_sb = cst.tile([P, CC], f32, tag="g1")
            nc.sync.dma_start(g1_sb[:], ln1_g_d.rearrange("(o p) -> p o", p=P))
            lb1_sb = cst.tile([P, CC], f32, tag="lb1")
            nc.sync.dma_start(lb1_sb[:], ln1_b_d.rearrange("(o p) -> p o", p=P))
            g2_sb = cst.tile([P, CC], f32, tag="g2")
            nc.sync.dma_start(g2_sb[:], ln2_g_d.rearrange("(o p) -> p o", p=P))
            lb2_sb = cst.tile([P, CC], f32, tag="lb2")
            nc.sync.dma_start(lb2_sb[:], ln2_b_d.rearrange("(o p) -> p o", p=P))

            bq_sb = cst.tile([P, CC], f32, tag="bq")
            nc.sync.dma_start(
                bq_sb[:], bq_d.rearrange("h d -> (h d)").rearrange("(o p) -> p o", p=P))
            bk_sb = cst.tile([P, CC], f32, tag="bk")
            nc.sync.dma_start(
                bk_sb[:], bk_d.rearrange("h d -> (h d)").rearrange("(o p) -> p o", p=P))
            bvrow = cst.tile([1, C], f32r, tag="bvrow")
            nc.sync.dma_start(bvrow[:], bv_d.rearrange("h d -> (h d)")[None])
            bp_sb = cst.tile([P, CC], f32, tag="bp")
            nc.sync.dma_start(bp_sb[:], bp_d.rearrange("(o p) -> p o", p=P))
            b1f_sb = cst.tile([P, FC], f32, tag="b1f")
            nc.sync.dma_start(b1f_sb[:], b1_d.rearrange("(o p) -> p o", p=P))
            b2_sb = cst.tile([P, CC], f32, tag="b2")
            nc.sync.dma_start(b2_sb[:], b2_d.rearrange("(o p) -> p o", p=P))

            eps_sb = cst.tile([P, 1], f32, tag="eps")
            nc.gpsimd.memset(eps_sb[:], EPS)

            ident = cst.tile([P, P], f32, tag="ident")
            make_identity(nc, ident[:])

            ones_col = cst.tile([1, P], f32r, tag="ones_col")
            nc.vector.tensor_scalar(ones_col[:], ident[0:1, :], 0.0, 1.0,
                                    OP.mult, OP.add)

            # V-hat padding pattern: col 0 -> 1.0 (denominator ones), rest 0
            vpad = cst.tile([P, 64], f32r, tag="vpad")
            nc.vector.tensor_scalar(vpad[:], ident[:, 0:64], 0.0, 0.0,
                                    OP.mult, OP.add)
            nc.vector.tensor_scalar(vpad[:, 0:1], ident[:, 0:1], 0.0, 1.0,
                                    OP.mult, OP.add)

            # indicator I6[h, c'] = 1 where c'//64 == h (for denom broadcast)
            # I6[h, c'] = 1 where 64h <= c' < 64h+64, built with two
            # affine selects (engine partition bases must be 32-aligned,
            # so no per-row memsets)
            I6f = cst.tile([H, C], f32, tag="I6f")
            nc.gpsimd.memset(I6f[:], 1.0)
            nc.gpsimd.affine_select(
                out=I6f[:], in_=I6f[:], compare_op=OP.is_ge, fill=0.0,
                base=0, pattern=[[1, C]], channel_multiplier=-64)
            nc.gpsimd.affine_select(
                out=I6f[:], in_=I6f[:], compare_op=OP.is_ge, fill=0.0,
                base=63, pattern=[[-1, C]], channel_multiplier=64)
            I6 = cst.tile([H, C], f32r, tag="I6")
            nc.vector.tensor_copy(I6[:], I6f[:])

            # causal masks, multiplicative; each [P, TP] = [tri(b0) | tri(b1)]
            # mask_sc[p, 256*bb + tl] = 1 if (p + 128*sc) <= tl else 0
            masks = []
            for sc in range(2):
                mk = cst.tile([P, TP], f32, tag=f"mask{sc}")
                for bb in range(2):
                    half = mk[:, 256 * bb:256 * bb + 256]
                    nc.gpsimd.memset(half, 1.0)
                    nc.gpsimd.affine_select(
                        out=half, in_=half,
                        compare_op=OP.is_ge, fill=0.0,
                        base=-128 * sc, pattern=[[1, 256]], channel_multiplier=-1)
                masks.append(mk)

            # ---------- per-pair loop ----------
            for pr in range(n_pairs):
                x_view = x_d[2 * pr:2 * pr + 2].rearrange("b (o p) c -> p (b o) c", p=P)
                y_view = y_d[2 * pr:2 * pr + 2].rearrange("b (o p) c -> p (b o) c", p=P)

                x_tok = p2.tile([P, 4, C], f32, tag="x_tok")
                nc.sync.dma_start(x_tok[:], x_view)

                def layernorm_tokens(src, g_sb, lb_sb, dstT, tagp):
                    """src: [P,4,C] token-major f32. Writes dstT [P,CC,TP] f32r
                    feature-major with gain/bias folded into the copy."""
                    s = p1.tile([P, 4], f32, tag=f"{tagp}_s")
                    nc.vector.tensor_reduce(s[:], src[:], mybir.AxisListType.X, OP.add)
                    sq = p1.tile([P, 4, C], f32, tag="sq_scratch")
                    nc.scalar.square(sq[:], src[:])
                    s2 = p1.tile([P, 4], f32, tag=f"{tagp}_s2")
                    nc.vector.tensor_reduce(s2[:], sq[:], mybir.AxisListType.X, OP.add)
                    mu = p1.tile([P, 4], f32, tag=f"{tagp}_mu")
                    nc.vector.tensor_scalar_mul(mu[:], s[:], 1.0 / C)
                    var = p1.tile([P, 4], f32, tag=f"{tagp}_var")
                    # var = s2/C - mu^2
                    nc.vector.tensor_scalar_mul(var[:], s2[:], 1.0 / C)
                    mu2 = p1.tile([P, 4], f32, tag=f"{tagp}_mu2")
                    nc.vector.tensor_tensor(mu2[:], mu[:], mu[:], OP.mult)
                    nc.vector.tensor_tensor(var[:], var[:], mu2[:], OP.subtract)
                    sd = p1.tile([P, 4], f32, tag=f"{tagp}_sd")
                    nc.scalar.activation(sd[:], var[:], AF.Sqrt, bias=eps_sb[:])
                    rs = p1.tile([P, 4], f32, tag=f"{tagp}_rs")
                    nc.vector.reciprocal(rs[:], sd[:])
                    murs = p1.tile([P, 4], f32, tag=f"{tagp}_murs")
                    nc.vector.tensor_tensor(murs[:], mu[:], rs[:], OP.mult)
                    htok = p1.tile([P, 4, C], f32, tag="htok", name="htok")
                    for so in range(4):
                        nc.vector.tensor_scalar(
                            htok[:, so], src[:, so], rs[:, so:so + 1],
                            murs[:, so:so + 1], OP.mult, OP.subtract)
                    for c in range(CC):
                        tp = psum()
                        for so in range(4):
                            nc.tensor.transpose(
                                tp[:, P * so:P * so + P],
                                htok[:, so, P * c:P * c + P], ident[:])
                        nc.vector.tensor_scalar(
                            dstT[:, c], tp[:], g_sb[:, c:c + 1], lb_sb[:, c:c + 1],
                            OP.mult, OP.add)

                h1T = p2.tile([P, CC, TP], f32r, tag="h1T")
                layernorm_tokens(x_tok, g1_sb, lb1_sb, h1T, "ln1")

                # ---- Q^T, K^T (feature-major) ----
                QT = p1.tile([P, CC, TP], f32r, tag="QT")
                KT = p1.tile([P, CC, TP], f32r, tag="KT")
                for (W_sb, b_sb, dst) in ((Wq_sb, bq_sb, QT), (Wk_sb, bk_sb, KT)):
                    for mo in range(CC):
                        ps = psum()
                        for c in range(CC):
                            nc.tensor.matmul(
                                ps[:], W_sb[:, c, P * mo:P * mo + P], h1T[:, c],
                                start=(c == 0), stop=(c == CC - 1))
                        nc.scalar.activation(dst[:, mo], ps[:], AF.Identity,
                                             bias=b_sb[:, mo:mo + 1])

                if debug_outputs and pr == 0:
                    nc.sync.dma_start(dbg["QT0"][:], QT[:, 0].bitcast(f32))
                    nc.sync.dma_start(dbg["KT0"][:], KT[:, 0].bitcast(f32))
                    nc.sync.dma_start(dbg["h1T0"][:], h1T[:, 0].bitcast(f32))
                # ---- V (token-major), ones column for denominators ----
                V_sb = p1.tile([P, 4, H, P], f32r, tag="V_sb")
                nc.vector.tensor_copy(
                    V_sb[:, :, :, 64:128],
                    vpad[:, None, None, :].to_broadcast((P, 4, H, 64)))
                for to in range(4):
                    ps = psum(w=C)
                    for c in range(CC):
                        nc.tensor.matmul(
                            ps[:], h1T[:, c, P * to:P * to + P], Wv_sb[:, c],
                            start=(c == 0), stop=False)
                    nc.tensor.matmul(ps[:], ones_col[:], bvrow[:],
                                     start=False, stop=True)
                    nc.scalar.activation(
                        V_sb[:, to, :, 0:64],
                        ps[:].rearrange("p (h d) -> p h d", h=H),
                        AF.Copy)

                if debug_outputs and pr == 0:
                    nc.sync.dma_start(
                        dbg["V0"][:],
                        V_sb[:, 0].bitcast(f32).rearrange("p h d -> p (h d)")[:, 0:TP])
                # ---- attention per head ----
                AVT = p1.tile([P, CC, TP], f32r, tag="fm_r", name="AVT")
                den6 = p1.tile([H, TP], f32, tag="den6")
                den_dr = drp.tile([H, TP], f32, tag="den_dr", name="den_dr")
                for h in range(H):
                    mo, half = h // 2, h % 2
                    rows = slice(64 * half, 64 * half + 64)
                    Es = []
                    for sc in range(2):
                        sps = psum()
                        for bb in range(2):
                            cols = slice(256 * bb, 256 * bb + 256)
                            nc.tensor.matmul(
                                sps[:, cols],
                                QT[rows, mo, 256 * bb + 128 * sc:
                                   256 * bb + 128 * sc + 128],
                                KT[rows, mo, cols],
                                start=True, stop=True)
                        e = p1.tile([P, TP], f32r, tag=f"E{sc}")
                        nc.scalar.activation(e[:], sps[:], AF.Exp, scale=SCALE)
                        nc.vector.tensor_tensor(e[:], e[:].bitcast(f32), masks[sc][:], OP.mult)
                        Es.append(e)
                    if debug_outputs and pr == 0 and h == 0:
                        nc.sync.dma_start(dbg["E0"][:], Es[0][:].bitcast(f32))
                    avps = psum()
                    for bb in range(2):
                        cols = slice(256 * bb, 256 * bb + 256)
                        for sc in range(2):
                            nc.tensor.matmul(
                                avps[:, cols], V_sb[:, 2 * bb + sc, h, :],
                                Es[sc][:, cols], start=(sc == 0), stop=(sc == 1))
                    nc.scalar.activation(AVT[rows, mo, :], avps[0:64, :], AF.Copy)
                    dstage = p1.tile([P, TP], f32, tag="den_stage", name="dstage")
                    nc.vector.tensor_copy(dstage[64:65, :], avps[64:65, :])
                    nc.sync.dma_start(den_dr[h:h + 1, :], dstage[64:65, :])

                nc.sync.dma_start(den6[:], den_dr[:])
                if debug_outputs and pr == 0:
                    nc.sync.dma_start(dbg["den6"][:], den6[:].bitcast(f32))
                recip6 = p1.tile([H, TP], f32r, tag="recip6")
                with nc.allow_low_precision(reason="softmax denom reciprocal to f32r"):
                    nc.vector.reciprocal(recip6[:], den6[:])
                for mo in range(CC):
                    rps = psum()
                    nc.tensor.matmul(rps[:], I6[:, P * mo:P * mo + P], recip6[:],
                                     start=True, stop=True)
                    if debug_outputs and pr == 0 and mo == 0:
                        rps_sb = p1.tile([P, TP], f32, tag="rps_dbg")
                        nc.vector.tensor_copy(rps_sb[:], rps[:])
                        nc.sync.dma_start(dbg["rps0"][:], rps_sb[:])
                    nc.vector.tensor_tensor(AVT[:, mo], AVT[:, mo].bitcast(f32), rps[:], OP.mult)
                if debug_outputs and pr == 0:
                    for mo in range(CC):
                        nc.sync.dma_start(dbg[f"AVT{mo}"][:], AVT[:, mo].bitcast(f32))

                # ---- proj + residual ----
                proj_sb = p1.tile([P, CC, TP], f32, tag="fm_f32", name="proj_sb")
                for mo in range(CC):
                    ps = psum()
                    for c in range(CC):
                        nc.tensor.matmul(
                            ps[:], Wp_sb[:, c, P * mo:P * mo + P], AVT[:, c],
                            start=(c == 0), stop=(c == CC - 1))
                    nc.vector.tensor_scalar(proj_sb[:, mo], ps[:],
                                            bp_sb[:, mo:mo + 1], None, OP.add)
                if debug_outputs and pr == 0:
                    nc.sync.dma_start(dbg["proj0"][:], proj_sb[:, 0])
                out1_tok = p1.tile([P, 4, C], f32, tag="out1_tok")
                for so in range(4):
                    tp = psum(w=C)
                    for mo in range(CC):
                        nc.tensor.transpose(
                            tp[:, P * mo:P * mo + P],
                            proj_sb[:, mo, P * so:P * so + P], ident[:])
                    nc.vector.tensor_tensor(out1_tok[:, so], tp[:], x_tok[:, so],
                                            OP.add)

                if debug_outputs and pr == 0:
                    nc.sync.dma_start(dbg["out1"][:],
                                      out1_tok[:].rearrange("p a c -> p (a c)"))
                # ---- LN2 + FFN ----
                h2T = p1.tile([P, CC, TP], f32r, tag="fm_r", name="h2T")
                layernorm_tokens(out1_tok, g2_sb, lb2_sb, h2T, "ln2")

                FF_sb = p1.tile([P, FC, TP], f32r, tag="FF_sb")
                for fo in range(FC):
                    ps = psum()
                    for c in range(CC):
                        nc.tensor.matmul(
                            ps[:], W1_sb[:, c, P * fo:P * fo + P], h2T[:, c],
                            start=(c == 0), stop=(c == CC - 1))
                    nc.scalar.activation(FF_sb[:, fo], ps[:], AF.Relu,
                                         bias=b1f_sb[:, fo:fo + 1])
                g_sb = p1.tile([P, CC, TP], f32, tag="fm_f32", name="g_sb")
                for mo in range(CC):
                    ps = psum()
                    for fo in range(FC):
                        nc.tensor.matmul(
                            ps[:], W2_sb[:, fo, P * mo:P * mo + P], FF_sb[:, fo],
                            start=(fo == 0), stop=(fo == FC - 1))
                    nc.vector.tensor_scalar(g_sb[:, mo], ps[:],
                                            b2_sb[:, mo:mo + 1], None, OP.add)

                y_tok = p2.tile([P, 4, C], f32, tag="y_tok")
                for so in range(4):
                    tp = psum(w=C)
                    for mo in range(CC):
                        nc.tensor.transpose(
                            tp[:, P * mo:P * mo + P],
                            g_sb[:, mo, P * so:P * so + P], ident[:])
                    nc.vector.tensor_tensor(y_tok[:, so], tp[:], out1_tok[:, so],
                                            OP.add)
                nc.sync.dma_start(y_view, y_tok[:])

    nc.compile()
    return nc


_NC_CACHE = {}


def kernel(_run_kwargs=None, **inputs) -> np.ndarray:
    run_kwargs = _run_kwargs or {}
    x = np.ascontiguousarray(np.asarray(inputs["x"], dtype=np.float32))
    weights = {k: np.ascontiguousarray(np.asarray(v, dtype=np.float32))
               for k, v in inputs.items() if k != "x"}

    if "nc" not in _NC_CACHE:
        _NC_CACHE["nc"] = build_nc()
    nc = _NC_CACHE["nc"]

    in_maps = []
    for c in range(N_CORES):
        m = {"x": x[c * B_LOCAL:(c + 1) * B_LOCAL]}
        m.update(weights)
        in_maps.append(m)

    res = run_bass_kernel_spmd(nc, in_maps, core_ids=list(range(N_CORES)), **run_kwargs)
    y = np.concatenate([r["y"] for r in res.results], axis=0)
    kernel.last_result = res
    return y
